# revision 2
# baseline (speedup 1.0000x reference)
"""GQA (grouped-query attention) forward kernel for 8 TRN2 NeuronCores.

Sharding: 8 cores = 2 (batch) x 4 (kv-head groups). Each core computes the
full attention for one batch element and one kv head (with its 4 query
heads), plus its slice of the row-parallel Wo projection; the host sums the
4 partial outputs per batch element.

Self-contained: hardcodes all shapes; takes full unsharded inputs.
"""
import math

import ml_dtypes
import numpy as np

import concourse.bass as bass  # noqa: F401  (bass types used via bacc)
import concourse.mybir as mybir
import concourse.tile as tile
from concourse import bacc
from concourse.bass_utils import run_bass_kernel_spmd

B, T, C = 2, 2048, 2048
H, KV, D = 16, 4, 128
R = H // KV            # query heads per kv head (per core)
P = 128                # partitions
CCH = C // P           # 16 contraction chunks
TB = 512               # T block for moving operands
NTB = T // TB          # 4
KT = T // P            # 16 key tiles of 128
SCALE = 1.0 / math.sqrt(D)

BF16 = ml_dtypes.bfloat16
dt = mybir.dt


def _emit(nc, tc, tensors):
    xT, wq, wk, wv, wo, cosT, sinT, masks, rotm, ident, ones1, bones, y = tensors

    import contextlib
    with contextlib.ExitStack() as ctx:
        cpool = ctx.enter_context(tc.tile_pool(name="const", bufs=1))
        xpool = ctx.enter_context(tc.tile_pool(name="xt", bufs=4))
        qkpool = ctx.enter_context(tc.tile_pool(name="qk", bufs=1))
        rpool = ctx.enter_context(tc.tile_pool(name="ropetmp", bufs=3))
        epool = ctx.enter_context(tc.tile_pool(name="exps", bufs=18))
        apool = ctx.enter_context(tc.tile_pool(name="attnsb", bufs=2))
        ypool = ctx.enter_context(tc.tile_pool(name="yout", bufs=3))
        pp = ctx.enter_context(tc.tile_pool(name="pp", bufs=3, space="PSUM"))
        psc = ctx.enter_context(tc.tile_pool(name="psc", bufs=2, space="PSUM"))
        ppo = ctx.enter_context(tc.tile_pool(name="ppo", bufs=2, space="PSUM"))
        pss = ctx.enter_context(tc.tile_pool(name="pss", bufs=1, space="PSUM"))

        # ---- resident constants -> SBUF ----
        wq_sb = cpool.tile([P, CCH * R * P], dt.bfloat16, tag="wq")
        nc.sync.dma_start(out=wq_sb, in_=wq[:, :])
        wk_sb = cpool.tile([P, CCH * P], dt.bfloat16, tag="wk")
        nc.sync.dma_start(out=wk_sb, in_=wk[:, :])
        wv_sb = cpool.tile([P, CCH * P], dt.bfloat16, tag="wv")
        nc.sync.dma_start(out=wv_sb, in_=wv[:, :])
        wo_sb = cpool.tile([P, R * C], dt.bfloat16, tag="wo")
        nc.sync.dma_start(out=wo_sb, in_=wo[:, :])
        cos_sb = cpool.tile([P, T], dt.float32, tag="cos")
        nc.sync.dma_start(out=cos_sb, in_=cosT[:, :])
        sin_sb = cpool.tile([P, T], dt.float32, tag="sin")
        nc.sync.dma_start(out=sin_sb, in_=sinT[:, :])
        mask_sb = cpool.tile([P, R * TB], dt.bfloat16, tag="masks")
        nc.sync.dma_start(out=mask_sb, in_=masks[:, :])
        rot_sb = cpool.tile([P, P], dt.bfloat16, tag="rotm")
        nc.sync.dma_start(out=rot_sb, in_=rotm[:, :])
        id_sb = cpool.tile([P, P], dt.bfloat16, tag="ident")
        nc.sync.dma_start(out=id_sb, in_=ident[:, :])
        ones_sb = cpool.tile([P, 1], dt.bfloat16, tag="ones1")
        nc.sync.dma_start(out=ones_sb, in_=ones1[:, :])
        bones_sb = cpool.tile([1, P], dt.bfloat16, tag="bones")
        nc.sync.dma_start(out=bones_sb, in_=bones[:, :])

        # ---- persistent activations ----
        qrope = [qkpool.tile([P, T], dt.bfloat16, tag=f"qrope{h}", name=f"qrope{h}")
                 for h in range(R)]
        krope = qkpool.tile([P, T], dt.bfloat16, tag="krope", name="krope")
        vnat = qkpool.tile([P, KT * P], dt.bfloat16, tag="vnat", name="vnat")
        aout = [qkpool.tile([P, T], dt.bfloat16, tag=f"aout{h}", name=f"aout{h}")
                for h in range(R)]

        def rope_one(p_src, src_lo, dst, tb):
            """p_src[: , src_lo:src_lo+TB] (psum f32) -> dst[:, tb*TB:+TB] roped bf16."""
            ts = slice(tb * TB, (tb + 1) * TB)
            s_sb = rpool.tile([P, TB], dt.bfloat16, tag="s_sb", name="s_sb")
            nc.scalar.copy(s_sb, p_src[:, src_lo:src_lo + TB])
            rot_ps = pp.tile([P, TB], dt.float32, tag="ps", name="rot_ps")
            nc.tensor.matmul(rot_ps, rot_sb, s_sb, start=True, stop=True)
            m1 = rpool.tile([P, TB], dt.bfloat16, tag="m1", name="m1")
            nc.vector.tensor_mul(m1, s_sb, cos_sb[:, ts])
            m2 = rpool.tile([P, TB], dt.bfloat16, tag="m2", name="m2")
            nc.vector.tensor_mul(m2, rot_ps, sin_sb[:, ts])
            nc.vector.tensor_add(dst[:, ts], m1, m2)

        for tb in range(NTB):
            ts = slice(tb * TB, (tb + 1) * TB)
            # ---- projections for this T block (two passes of 3 PSUM groups) ----
            pA = [pp.tile([P, TB], dt.float32, tag="ps", name=f"pA{i}_{tb}")
                  for i in range(3)]
            for c in range(CCH):
                xt = xpool.tile([P, TB], dt.bfloat16, tag="xt", name="xt")
                nc.sync.dma_start(out=xt, in_=xT[:, c * T + tb * TB:c * T + (tb + 1) * TB])
                fl = dict(start=(c == 0), stop=(c == CCH - 1))
                for j in range(3):  # q0 q1 q2
                    nc.tensor.matmul(pA[j], wq_sb[:, c * 512 + j * P:c * 512 + (j + 1) * P],
                                     xt, **fl)
            for j in range(3):
                rope_one(pA[j], 0, qrope[j], tb)
            pB = [pp.tile([P, TB], dt.float32, tag="ps", name=f"pB{i}_{tb}")
                  for i in range(3)]
            for c in range(CCH):
                xt = xpool.tile([P, TB], dt.bfloat16, tag="xt", name="xt")
                nc.sync.dma_start(out=xt, in_=xT[:, c * T + tb * TB:c * T + (tb + 1) * TB])
                fl = dict(start=(c == 0), stop=(c == CCH - 1))
                nc.tensor.matmul(pB[0], wq_sb[:, c * 512 + 3 * P:c * 512 + 4 * P], xt, **fl)
                nc.tensor.matmul(pB[1], wk_sb[:, c * P:(c + 1) * P], xt, **fl)
                nc.tensor.matmul(pB[2], wv_sb[:, c * P:(c + 1) * P], xt, **fl)
            rope_one(pB[0], 0, qrope[3], tb)
            rope_one(pB[1], 0, krope, tb)
            # v: cast then transpose to natural [Tk, D] tiles
            vT_sb = rpool.tile([P, TB], dt.bfloat16, tag="vT", name="vT_sb")
            nc.scalar.copy(vT_sb, pB[2])
            for i in range(TB // P):
                kt = tb * (TB // P) + i
                vt_ps = pp.tile([P, P], dt.bfloat16, tag="ps", name="vt_ps")
                nc.tensor.transpose(vt_ps, vT_sb[:, i * P:(i + 1) * P], id_sb)
                nc.scalar.copy(vnat[:, kt * P:(kt + 1) * P], vt_ps)

            # ---- attention for q block qb = tb (all 4 heads) ----
            qb = tb
            kt_max = (qb + 1) * (TB // P)
            qs = slice(qb * TB, (qb + 1) * TB)
            for h in range(R):
                es_tiles = []
                for kt in range(kt_max):
                    sc_ps = psc.tile([P, TB], dt.float32, tag="sc", name="sc_ps")
                    nc.tensor.matmul(sc_ps, krope[:, kt * P:(kt + 1) * P],
                                     qrope[h][:, qs], start=True, stop=True)
                    es = epool.tile([P, TB], dt.bfloat16, tag="es", name="es")
                    nc.scalar.activation(es, sc_ps,
                                         mybir.ActivationFunctionType.Exp,
                                         scale=SCALE)
                    j = kt - qb * (TB // P)
                    if j >= 0:  # diagonal block: causal mask
                        nc.vector.tensor_mul(es, es, mask_sb[:, j * TB:(j + 1) * TB])
                    es_tiles.append(es)
                po = ppo.tile([P, TB], dt.float32, tag="po", name="po")
                for kt in range(kt_max):
                    nc.tensor.matmul(po, vnat[:, kt * P:(kt + 1) * P], es_tiles[kt],
                                     start=(kt == 0), stop=(kt == kt_max - 1))
                ssum = pss.tile([1, TB], dt.float32, tag="ss", name="ssum")
                for kt in range(kt_max):
                    nc.tensor.matmul(ssum, ones_sb, es_tiles[kt],
                                     start=(kt == 0), stop=(kt == kt_max - 1))
                rcp = apool.tile([1, TB], dt.bfloat16, tag="rcp", name="rcp")
                with nc.allow_low_precision(reason="bf16 softmax denom is plenty"):
                    nc.vector.reciprocal(rcp, ssum)
                bc_ps = pss.tile([P, TB], dt.float32, tag="ss", name="bc_ps")
                nc.tensor.matmul(bc_ps, bones_sb, rcp, start=True, stop=True)
                bc_sb = apool.tile([P, TB], dt.float32, tag="bc", name="bc_sb")
                nc.scalar.copy(bc_sb, bc_ps)
                nc.vector.tensor_mul(aout[h][:, qs], po, bc_sb)

            # ---- Wo projection for the freshly finished T rows ----
            for tsub in range(qb * (TB // P), (qb + 1) * (TB // P)):
                for nb in range(NTB):
                    yp = pp.tile([P, TB], dt.float32, tag="ps", name="yp")
                    for h in range(R):
                        nc.tensor.matmul(
                            yp,
                            aout[h][:, tsub * P:(tsub + 1) * P],
                            wo_sb[:, h * C + nb * TB:h * C + (nb + 1) * TB],
                            start=(h == 0), stop=(h == R - 1))
                    y_sb = ypool.tile([P, TB], dt.float32, tag="ysb", name="y_sb")
                    nc.scalar.copy(y_sb, yp)
                    nc.sync.dma_start(
                        out=y[tsub * P:(tsub + 1) * P, nb * TB:(nb + 1) * TB],
                        in_=y_sb)


def build():
    nc = bacc.Bacc("TRN2", target_bir_lowering=False, debug=False, num_devices=8)
    xT = nc.dram_tensor("xT", [P, CCH * T], dt.bfloat16, kind="ExternalInput")
    wq = nc.dram_tensor("wq", [P, CCH * R * P], dt.bfloat16, kind="ExternalInput")
    wk = nc.dram_tensor("wk", [P, CCH * P], dt.bfloat16, kind="ExternalInput")
    wv = nc.dram_tensor("wv", [P, CCH * P], dt.bfloat16, kind="ExternalInput")
    wo = nc.dram_tensor("wo", [P, R * C], dt.bfloat16, kind="ExternalInput")
    cosT = nc.dram_tensor("cosT", [P, T], dt.float32, kind="ExternalInput")
    sinT = nc.dram_tensor("sinT", [P, T], dt.float32, kind="ExternalInput")
    masks = nc.dram_tensor("masks", [P, R * TB], dt.bfloat16, kind="ExternalInput")
    rotm = nc.dram_tensor("rotm", [P, P], dt.bfloat16, kind="ExternalInput")
    ident = nc.dram_tensor("ident", [P, P], dt.bfloat16, kind="ExternalInput")
    ones1 = nc.dram_tensor("ones1", [P, 1], dt.bfloat16, kind="ExternalInput")
    bones = nc.dram_tensor("bones", [1, P], dt.bfloat16, kind="ExternalInput")
    y = nc.dram_tensor("y", [T, C], dt.float32, kind="ExternalOutput")

    tensors = (xT, wq, wk, wv, wo, cosT, sinT, masks, rotm, ident, ones1, bones, y)
    with tile.TileContext(nc) as tc:
        _emit(nc, tc, tensors)
    nc.compile()
    return nc


def _chunk128(a):
    """[n*128, m] -> [128, n*m] with row r holding chunks a[c*128+r, :]."""
    n = a.shape[0] // P
    return np.ascontiguousarray(
        a.reshape(n, P, a.shape[1]).transpose(1, 0, 2).reshape(P, n * a.shape[1]))


def _host_consts():
    half = D // 2
    theta = 1.0 / (10000.0 ** (np.arange(half, dtype=np.float32) / half))
    pos = np.arange(T, dtype=np.float32)
    freqs = pos[:, None] * theta[None, :]
    freqs = np.concatenate([freqs, freqs], axis=-1)          # (T, D)
    cosT = np.ascontiguousarray(np.cos(freqs).T.astype(np.float32))   # (D, T)
    sinT = np.ascontiguousarray(np.sin(freqs).T.astype(np.float32))

    # masks[p, j*TB + f] = 1 if p <= f - 128*j else 0  (post-exp causal mask)
    pidx = np.arange(P)[:, None]
    fidx = np.arange(TB)[None, :]
    masks = np.concatenate(
        [(pidx <= (fidx - P * j)).astype(np.float32) for j in range(R)], axis=1)

    rotm = np.zeros((P, P), dtype=np.float32)
    rotm[np.arange(half), half + np.arange(half)] = 1.0   # upper-right +I
    rotm[half + np.arange(half), np.arange(half)] = -1.0  # lower-left  -I

    ident = np.eye(P, dtype=np.float32)
    ones1 = np.ones((P, 1), dtype=np.float32)
    bones = np.ones((1, P), dtype=np.float32)
    return {
        "cosT": cosT,
        "sinT": sinT,
        "masks": masks.astype(BF16),
        "rotm": rotm.astype(BF16),
        "ident": ident.astype(BF16),
        "ones1": ones1.astype(BF16),
        "bones": bones.astype(BF16),
    }


_NC_CACHE = [None]


def run(inputs, trace=False, tmpdir=None):
    x, Wq, Wk, Wv, Wo = (np.asarray(inputs[k]) for k in ("x", "Wq", "Wk", "Wv", "Wo"))
    consts = _host_consts()

    xT_r = [None] * B
    for b in range(B):
        xT_r[b] = _chunk128(np.ascontiguousarray(x[b].T).astype(BF16))

    in_maps = []
    for b in range(B):
        for g in range(KV):
            wq_g = _chunk128(Wq[:, g * R * D:(g + 1) * R * D].astype(BF16))
            wk_g = _chunk128(Wk[:, g * D:(g + 1) * D].astype(BF16))
            wv_g = _chunk128(Wv[:, g * D:(g + 1) * D].astype(BF16))
            wo_g = _chunk128(Wo[g * R * D:(g + 1) * R * D, :].astype(BF16))
            m = {"xT": xT_r[b], "wq": wq_g, "wk": wk_g, "wv": wv_g, "wo": wo_g}
            m.update(consts)
            in_maps.append(m)

    if _NC_CACHE[0] is None:
        _NC_CACHE[0] = build()
    nc = _NC_CACHE[0]

    res = run_bass_kernel_spmd(nc, in_maps, core_ids=list(range(8)),
                               trace=trace, tmpdir=tmpdir)
    out = np.zeros((B, T, C), dtype=np.float32)
    for core in range(8):
        out[core // KV] += res.results[core]["y"]
    return out, res


def kernel(x, Wq, Wk, Wv, Wo):
    out, _ = run({"x": x, "Wq": Wq, "Wk": Wk, "Wv": Wv, "Wo": Wo})
    return out


# revision 6
# speedup vs baseline: 1.3883x; 1.3883x over previous
"""GQA (grouped-query attention) forward kernel for 8 TRN2 NeuronCores.

Sharding: 8 cores = 2 (batch) x 4 (kv-head groups). Each core computes the
full attention for one batch element and one kv head (with its 4 query
heads), plus its slice of the row-parallel Wo projection; the host sums the
4 partial outputs per batch element.

Self-contained: hardcodes all shapes; takes full unsharded inputs.
"""
import math

import ml_dtypes
import numpy as np

import concourse.bass as bass  # noqa: F401  (bass types used via bacc)
import concourse.mybir as mybir
import concourse.tile as tile
from concourse import bacc
from concourse.bass_utils import run_bass_kernel_spmd

B, T, C = 2, 2048, 2048
H, KV, D = 16, 4, 128
R = H // KV            # query heads per kv head (per core)
P = 128                # partitions
CCH = C // P           # 16 contraction chunks
TB = 512               # T block (attention q-blocks and projection blocks)
NTB = T // TB          # 4
KT = T // P            # 16 key tiles of 128
SCALE = 1.0 / math.sqrt(D)

BF16 = ml_dtypes.bfloat16
dt = mybir.dt
F32R = dt.float32r


def _emit(nc, tc, tensors):
    (xT, wq, wk, wv, wo, cosT, sinT, tri, rotm, ident, onesf, bonesf, y) = tensors

    import contextlib
    with contextlib.ExitStack() as ctx:
        cpool = ctx.enter_context(tc.tile_pool(name="const", bufs=1))
        xpool = ctx.enter_context(tc.tile_pool(name="xt", bufs=17))
        qkpool = ctx.enter_context(tc.tile_pool(name="qk", bufs=1))
        rpool = ctx.enter_context(tc.tile_pool(name="ropetmp", bufs=4))
        epool = ctx.enter_context(tc.tile_pool(name="exps", bufs=18))
        apool = ctx.enter_context(tc.tile_pool(name="attnsb", bufs=3))
        ypool = ctx.enter_context(tc.tile_pool(name="yout", bufs=3))
        pp = ctx.enter_context(tc.tile_pool(name="pp", bufs=3, space="PSUM"))
        psc = ctx.enter_context(tc.tile_pool(name="psc", bufs=3, space="PSUM"))
        ppo = ctx.enter_context(tc.tile_pool(name="ppo", bufs=2, space="PSUM"))

        # ---- resident constants -> SBUF ----
        wq_sb = cpool.tile([P, CCH * R * P], dt.bfloat16, tag="wq")
        nc.sync.dma_start(out=wq_sb, in_=wq[:, :])
        wk_sb = cpool.tile([P, CCH * P], dt.bfloat16, tag="wk")
        nc.sync.dma_start(out=wk_sb, in_=wk[:, :])
        wv_sb = cpool.tile([P, CCH * P], dt.bfloat16, tag="wv")
        nc.sync.dma_start(out=wv_sb, in_=wv[:, :])
        wo_sb = cpool.tile([P, R * C], dt.bfloat16, tag="wo")
        nc.sync.dma_start(out=wo_sb, in_=wo[:, :])
        cos_sb = cpool.tile([P, T], dt.float32, tag="cos")
        nc.sync.dma_start(out=cos_sb, in_=cosT[:, :])
        sin_sb = cpool.tile([P, T], dt.float32, tag="sin")
        nc.sync.dma_start(out=sin_sb, in_=sinT[:, :])
        tri_sb = cpool.tile([P, P], dt.bfloat16, tag="tri")
        nc.sync.dma_start(out=tri_sb, in_=tri[:, :])
        rot_sb = cpool.tile([P, P], dt.bfloat16, tag="rotm")
        nc.sync.dma_start(out=rot_sb, in_=rotm[:, :])
        id_sb = cpool.tile([P, P], dt.bfloat16, tag="ident")
        nc.sync.dma_start(out=id_sb, in_=ident[:, :])
        of_sb = cpool.tile([P, 1], F32R, tag="onesf")
        nc.sync.dma_start(out=of_sb, in_=onesf[:, :])
        bf_sb = cpool.tile([1, P], F32R, tag="bonesf")
        nc.sync.dma_start(out=bf_sb, in_=bonesf[:, :])

        # ---- persistent activations ----
        qrope = [qkpool.tile([P, T], dt.bfloat16, tag=f"qrope{h}", name=f"qrope{h}")
                 for h in range(R)]
        krope = qkpool.tile([P, T], dt.bfloat16, tag="krope", name="krope")
        vnat = qkpool.tile([P, KT * P], dt.bfloat16, tag="vnat", name="vnat")
        aout = [qkpool.tile([P, T], dt.bfloat16, tag=f"aout{h}", name=f"aout{h}")
                for h in range(R)]

        def rope_one(p_src, dst, tb):
            """p_src (psum f32 [128, TB]) -> dst[:, tb*TB:+TB] roped bf16."""
            tsl = slice(tb * TB, (tb + 1) * TB)
            s_sb = rpool.tile([P, TB], dt.bfloat16, tag="s_sb", name="s_sb")
            nc.scalar.copy(s_sb, p_src)
            rot_ps = psc.tile([P, TB], dt.float32, tag="sc", name="rot_ps")
            nc.tensor.matmul(rot_ps, rot_sb, s_sb, start=True, stop=True)
            m1 = rpool.tile([P, TB], dt.bfloat16, tag="m1", name="m1")
            nc.vector.tensor_mul(m1, s_sb, cos_sb[:, tsl])
            m2 = rpool.tile([P, TB], dt.bfloat16, tag="m2", name="m2")
            nc.vector.tensor_mul(m2, rot_ps, sin_sb[:, tsl])
            nc.vector.tensor_add(dst[:, tsl], m1, m2)

        def attn_block(qb):
            kt_max = (qb + 1) * (TB // P)
            ndiag = TB // P
            for h in range(R):
                es_tiles = []
                acc = apool.tile([P, TB], F32R, tag="acc", name="acc")
                for kt in range(kt_max):
                    j = kt - qb * ndiag  # >=0 on the diagonal group
                    off = max(0, j) * P  # causal: columns [off, TB) only
                    w = TB - off
                    q0 = qb * TB + off
                    sc_ps = psc.tile([P, w], dt.float32, tag="sc", name="sc_ps")
                    nc.tensor.matmul(sc_ps, krope[:, kt * P:(kt + 1) * P],
                                     qrope[h][:, q0:q0 + w], start=True, stop=True)
                    es = epool.tile([P, w], dt.bfloat16, tag="es", name="es")
                    nc.scalar.activation(es, sc_ps,
                                         mybir.ActivationFunctionType.Exp,
                                         scale=SCALE)
                    if j >= 0:  # triangle mask on the first 128 columns
                        nc.gpsimd.tensor_mul(es[:, 0:P], es[:, 0:P], tri_sb)
                    es_tiles.append((es, off, w))
                    # interleaved row-sum accumulation on DVE
                    if kt == 0:
                        nc.vector.tensor_copy(acc, es)
                    else:
                        nc.vector.tensor_add(acc[:, off:], acc[:, off:], es)
                po = ppo.tile([P, TB], dt.float32, tag="po", name="po")
                for kt, (es, off, w) in enumerate(es_tiles):
                    nc.tensor.matmul(po[:, off:], vnat[:, kt * P:(kt + 1) * P], es,
                                     start=(kt == 0), stop=(kt == kt_max - 1),
                                     skip_group_check=True)
                # denominators: one cross-partition sum (f32r), then 1/x via exp(-ln)
                ssum = psc.tile([1, TB], dt.float32, tag="sc", name="ssum")
                nc.tensor.matmul(ssum, of_sb, acc, start=True, stop=True)
                lns = apool.tile([1, TB], dt.float32, tag="lns", name="lns")
                nc.scalar.activation(lns, ssum, mybir.ActivationFunctionType.Ln)
                nrm = apool.tile([1, TB], F32R, tag="nrm", name="nrm")
                nc.scalar.activation(nrm, lns, mybir.ActivationFunctionType.Exp,
                                     scale=-1.0)
                bc_ps = psc.tile([P, TB], dt.float32, tag="sc", name="bc_ps")
                nc.tensor.matmul(bc_ps, bf_sb, nrm, start=True, stop=True)
                bc_sb = apool.tile([P, TB], dt.float32, tag="bc", name="bc_sb")
                nc.any.tensor_copy(bc_sb, bc_ps)
                nc.vector.tensor_mul(aout[h][:, qb * TB:(qb + 1) * TB], po, bc_sb)

        def wo_block(qb):
            for tsub in range(qb * (TB // P), (qb + 1) * (TB // P)):
                for nb in range(NTB):
                    yp = ppo.tile([P, TB], dt.float32, tag="po", name="yp")
                    for h in range(R):
                        nc.tensor.matmul(
                            yp,
                            aout[h][:, tsub * P:(tsub + 1) * P],
                            wo_sb[:, h * C + nb * TB:h * C + (nb + 1) * TB],
                            start=(h == 0), stop=(h == R - 1))
                    y_sb = ypool.tile([P, TB], dt.float32, tag="ysb", name="y_sb")
                    nc.any.tensor_copy(y_sb, yp)
                    nc.sync.dma_start(
                        out=y[tsub * P:(tsub + 1) * P, nb * TB:(nb + 1) * TB],
                        in_=y_sb)

        for tb in range(NTB):
            ts0 = tb * TB
            # ---- projections: 16 resident x tiles, 6 sequential PSUM groups ----
            xts = []
            for c in range(CCH):
                xt = xpool.tile([P, TB], dt.bfloat16, tag="xt", name="xt")
                nc.sync.dma_start(out=xt, in_=xT[:, c * T + ts0:c * T + ts0 + TB])
                xts.append(xt)

            def proj_group(w_sb, col0):
                pj = pp.tile([P, TB], dt.float32, tag="pj", name="pj")
                for c in range(CCH):
                    nc.tensor.matmul(pj, w_sb[:, col0(c):col0(c) + P], xts[c],
                                     start=(c == 0), stop=(c == CCH - 1))
                return pj

            for jh in range(R):
                pj = proj_group(wq_sb, lambda c, jh=jh: c * 512 + jh * P)
                rope_one(pj, qrope[jh], tb)
            pj = proj_group(wk_sb, lambda c: c * P)
            rope_one(pj, krope, tb)
            pj = proj_group(wv_sb, lambda c: c * P)
            # v: cast then transpose to natural [Tk, D] tiles
            vT_sb = rpool.tile([P, TB], dt.bfloat16, tag="vT", name="vT_sb")
            nc.scalar.copy(vT_sb, pj)
            for i in range(TB // P):
                kt = (ts0 // P) + i
                vt_ps = psc.tile([P, P], dt.bfloat16, tag="sc", name="vt_ps")
                nc.tensor.transpose(vt_ps, vT_sb[:, i * P:(i + 1) * P], id_sb)
                nc.scalar.copy(vnat[:, kt * P:(kt + 1) * P], vt_ps)

            # ---- attention + output proj for this finished 512-block ----
            attn_block(tb)
            wo_block(tb)


def build():
    nc = bacc.Bacc("TRN2", target_bir_lowering=False, debug=False, num_devices=8)
    xT = nc.dram_tensor("xT", [P, CCH * T], dt.bfloat16, kind="ExternalInput")
    wq = nc.dram_tensor("wq", [P, CCH * R * P], dt.bfloat16, kind="ExternalInput")
    wk = nc.dram_tensor("wk", [P, CCH * P], dt.bfloat16, kind="ExternalInput")
    wv = nc.dram_tensor("wv", [P, CCH * P], dt.bfloat16, kind="ExternalInput")
    wo = nc.dram_tensor("wo", [P, R * C], dt.bfloat16, kind="ExternalInput")
    cosT = nc.dram_tensor("cosT", [P, T], dt.float32, kind="ExternalInput")
    sinT = nc.dram_tensor("sinT", [P, T], dt.float32, kind="ExternalInput")
    tri = nc.dram_tensor("tri", [P, P], dt.bfloat16, kind="ExternalInput")
    rotm = nc.dram_tensor("rotm", [P, P], dt.bfloat16, kind="ExternalInput")
    ident = nc.dram_tensor("ident", [P, P], dt.bfloat16, kind="ExternalInput")
    onesf = nc.dram_tensor("onesf", [P, 1], F32R, kind="ExternalInput")
    bonesf = nc.dram_tensor("bonesf", [1, P], F32R, kind="ExternalInput")
    y = nc.dram_tensor("y", [T, C], dt.float32, kind="ExternalOutput")

    tensors = (xT, wq, wk, wv, wo, cosT, sinT, tri, rotm, ident, onesf, bonesf, y)
    with tile.TileContext(nc) as tc:
        _emit(nc, tc, tensors)
    nc.compile()
    return nc


def _chunk128(a):
    """[n*128, m] -> [128, n*m] with row r holding chunks a[c*128+r, :]."""
    n = a.shape[0] // P
    return np.ascontiguousarray(
        a.reshape(n, P, a.shape[1]).transpose(1, 0, 2).reshape(P, n * a.shape[1]))


def _host_consts():
    half = D // 2
    theta = 1.0 / (10000.0 ** (np.arange(half, dtype=np.float32) / half))
    pos = np.arange(T, dtype=np.float32)
    freqs = pos[:, None] * theta[None, :]
    freqs = np.concatenate([freqs, freqs], axis=-1)          # (T, D)
    cosT = np.ascontiguousarray(np.cos(freqs).T.astype(np.float32))   # (D, T)
    sinT = np.ascontiguousarray(np.sin(freqs).T.astype(np.float32))

    pidx = np.arange(P)[:, None]
    fidx = np.arange(P)[None, :]
    tri = (pidx <= fidx).astype(np.float32)   # keep tk<=tq on diagonal 128-cols

    rotm = np.zeros((P, P), dtype=np.float32)
    rotm[np.arange(half), half + np.arange(half)] = 1.0   # upper-right +I
    rotm[half + np.arange(half), np.arange(half)] = -1.0  # lower-left  -I

    ident = np.eye(P, dtype=np.float32)
    return {
        "cosT": cosT,
        "sinT": sinT,
        "tri": tri.astype(BF16),
        "rotm": rotm.astype(BF16),
        "ident": ident.astype(BF16),
        "onesf": np.ones((P, 1), dtype=np.float32),
        "bonesf": np.ones((1, P), dtype=np.float32),
    }


_NC_CACHE = [None]


def run(inputs, trace=False, tmpdir=None):
    x, Wq, Wk, Wv, Wo = (np.asarray(inputs[k]) for k in ("x", "Wq", "Wk", "Wv", "Wo"))
    consts = _host_consts()

    xT_r = [None] * B
    for b in range(B):
        xT_r[b] = _chunk128(np.ascontiguousarray(x[b].T).astype(BF16))

    in_maps = []
    for b in range(B):
        for g in range(KV):
            wq_g = _chunk128(Wq[:, g * R * D:(g + 1) * R * D].astype(BF16))
            wk_g = _chunk128(Wk[:, g * D:(g + 1) * D].astype(BF16))
            wv_g = _chunk128(Wv[:, g * D:(g + 1) * D].astype(BF16))
            wo_g = _chunk128(Wo[g * R * D:(g + 1) * R * D, :].astype(BF16))
            m = {"xT": xT_r[b], "wq": wq_g, "wk": wk_g, "wv": wv_g, "wo": wo_g}
            m.update(consts)
            in_maps.append(m)

    if _NC_CACHE[0] is None:
        _NC_CACHE[0] = build()
    nc = _NC_CACHE[0]

    res = run_bass_kernel_spmd(nc, in_maps, core_ids=list(range(8)),
                               trace=trace, tmpdir=tmpdir)
    out = np.zeros((B, T, C), dtype=np.float32)
    for core in range(8):
        out[core // KV] += res.results[core]["y"]
    return out, res


def kernel(x, Wq, Wk, Wv, Wo):
    out, _ = run({"x": x, "Wq": Wq, "Wk": Wk, "Wv": Wv, "Wo": Wo})
    return out
